# revision 1
# baseline (speedup 1.0000x reference)
"""Trainium2 Bass kernel: fused segment_max + segment_softmax (GNN message passing).

Computes, for M [E, d] and dest [E] with N nodes:
  M_v = segment_max(M, dest, N)                         -> [N, d]
  w   = segment_softmax(M * BETA, dest)[:, None, :]     -> [E, 1, d]

Strategy: host groups edges by destination node (dest-based partitioning),
buckets nodes by degree (rounded up to a multiple of 4, padded with a large
negative value), and shards node groups evenly across the 8 NeuronCores so all
segment reductions are device-local. On device, nodes live one-per-partition
with their edges feature-major in the free dimension, so segment max/sum are
single Vector-engine reduces over the innermost axis and every DMA is a
contiguous 2D transfer.
"""

import numpy as np

import concourse.bacc as bacc
import concourse.tile as tile
from concourse import mybir
from concourse import bass_utils

BETA = 100.0
PAD_VAL = -1.0e30
N_CORES = 8
BUCKET_STEP = 4          # degree rounded up to multiple of this
TARGET_FREE = 8192       # target free-dim elements per super-tile (32 KiB/partition fp32)
ALIGN_NODES = N_CORES * 128  # bucket counts aligned to this so per-core tiles are full


def _build_plan(deg):
    """Group nodes into degree buckets.

    Returns list of (D, node_ids) where node_ids is an int64 array whose length
    is a multiple of N_CORES (padded with -1 = dummy node). All buckets except
    the last have length a multiple of ALIGN_NODES (via promotion of remainder
    nodes into the next bucket up), so per-core tiles use all 128 partitions.
    """
    N = deg.shape[0]
    nodes = np.flatnonzero(deg > 0)
    bucket_of = ((deg[nodes] + BUCKET_STEP - 1) // BUCKET_STEP) * BUCKET_STEP
    bucket_vals = np.unique(bucket_of)

    buckets = {}
    for D in bucket_vals:
        buckets[int(D)] = nodes[bucket_of == D]

    # Promote remainders upward so every bucket count is a multiple of
    # ALIGN_NODES; the last bucket keeps its remainder (padded to N_CORES).
    ds = sorted(buckets.keys())
    for i, D in enumerate(ds):
        if i == len(ds) - 1:
            break
        ids = buckets[D]
        rem = len(ids) % ALIGN_NODES
        if rem:
            nxt = ds[i + 1]
            buckets[nxt] = np.concatenate([ids[len(ids) - rem:], buckets[nxt]])
            buckets[D] = ids[: len(ids) - rem]

    plan = []
    for D in ds:
        ids = buckets[D]
        if len(ids) == 0:
            continue
        pad = (-len(ids)) % N_CORES
        if pad:
            ids = np.concatenate([ids, np.full(pad, -1, dtype=ids.dtype)])
        plan.append((int(D), ids))
    return plan


def _pack(M, dest, N):
    """Build per-core flat input arrays plus the metadata needed to unpack."""
    E, d = M.shape
    deg = np.bincount(dest, minlength=N)
    order = np.argsort(dest, kind="stable")
    starts = np.zeros(N + 1, dtype=np.int64)
    np.cumsum(deg, out=starts[1:])

    plan = _build_plan(deg)

    # Per-core flat layout: for each bucket (D, ids): per node [d, D]
    # feature-major. L = per-core element count.
    sizes = [(D, len(ids) // N_CORES) for D, ids in plan]
    L = sum(m8 * d * D for D, m8 in sizes)
    NV = sum(m8 for _, m8 in sizes)

    flat = np.full((N_CORES, L), PAD_VAL, dtype=np.float32)
    meta = []  # (D, ids [m], eids [m, D] int64, valid [m, D] bool, off, mvoff)
    off = 0
    mvoff = 0
    for D, ids in plan:
        m = len(ids)
        m8 = m // N_CORES
        safe_ids = np.where(ids >= 0, ids, 0)
        degs = np.where(ids >= 0, deg[safe_ids], 0)
        pos = starts[safe_ids][:, None] + np.arange(D)[None, :]
        valid = np.arange(D)[None, :] < degs[:, None]
        eids = order[np.minimum(pos, E - 1)]
        eids = np.where(valid, eids, 0)

        blk = M[eids]                      # [m, D, d]
        blk[~valid] = PAD_VAL
        blk = np.ascontiguousarray(blk.transpose(0, 2, 1))  # [m, d, D]
        span = m8 * d * D
        per_core = blk.reshape(N_CORES, span)
        flat[:, off:off + span] = per_core

        meta.append((D, ids, eids, valid, off, mvoff))
        off += span
        mvoff += m8
    assert off == L and mvoff == NV

    return flat, sizes, meta, L, NV, deg


def _emit_bucket_ops(nc, pools, m_in, w_out, mv_out, D, m8, off, mvoff, d):
    """Emit device ops for one degree bucket (per-core node count m8)."""
    big, small = pools
    A = max(1, TARGET_FREE // (d * D))
    f32 = mybir.dt.float32
    pos = 0
    while pos < m8:
        left = m8 - pos
        if left >= 128:
            a = min(A, left // 128)
            p = 128
            n_nodes = a * 128
        else:
            a = 1
            p = left
            n_nodes = left
        x = a * d          # node-feature columns in this tile
        fd = x * D         # free-dim elements in this tile

        base = off + pos * d * D
        mvbase = (mvoff + pos) * d

        mt = big.tile([128, fd], f32, tag="mt", name=f"mt_{D}_{pos}")
        nc.sync.dma_start(
            out=mt[:p, :],
            in_=m_in[base:base + p * fd].rearrange("(p x) -> p x", p=p),
        )
        mt3 = mt[:p, :].rearrange("p (x e) -> p x e", e=D)

        mx = small.tile([128, x], f32, tag="mx", name=f"mx_{D}_{pos}")
        nc.vector.tensor_reduce(
            out=mx[:p, :], in_=mt3, axis=mybir.AxisListType.X,
            op=mybir.AluOpType.max,
        )
        nc.sync.dma_start(
            out=mv_out[mvbase:mvbase + p * x].rearrange("(p x) -> p x", p=p),
            in_=mx[:p, :],
        )

        mx_b = mx[:p, :].unsqueeze(2).to_broadcast([p, x, D])
        nc.vector.tensor_tensor(
            out=mt3, in0=mt3, in1=mx_b, op=mybir.AluOpType.subtract,
        )
        nc.scalar.activation(
            out=mt[:p, :], in_=mt[:p, :],
            func=mybir.ActivationFunctionType.Exp, scale=BETA,
        )
        sm = small.tile([128, x], f32, tag="sm", name=f"sm_{D}_{pos}")
        nc.vector.tensor_reduce(
            out=sm[:p, :], in_=mt3, axis=mybir.AxisListType.X,
            op=mybir.AluOpType.add,
        )
        rc = small.tile([128, x], f32, tag="rc", name=f"rc_{D}_{pos}")
        nc.vector.reciprocal(out=rc[:p, :], in_=sm[:p, :])
        rc_b = rc[:p, :].unsqueeze(2).to_broadcast([p, x, D])
        nc.vector.tensor_tensor(
            out=mt3, in0=mt3, in1=rc_b, op=mybir.AluOpType.mult,
        )
        nc.sync.dma_start(
            out=w_out[base:base + p * fd].rearrange("(p x) -> p x", p=p),
            in_=mt[:p, :],
        )
        pos += n_nodes


def _build_bass(sizes, L, NV, d):
    nc = bacc.Bacc("TRN2", target_bir_lowering=False, debug=False,
                   num_devices=N_CORES)
    f32 = mybir.dt.float32
    m_in = nc.dram_tensor("m_in", [L], f32, kind="ExternalInput").ap()
    w_out = nc.dram_tensor("w_out", [L], f32, kind="ExternalOutput").ap()
    mv_out = nc.dram_tensor("mv_out", [NV * d], f32, kind="ExternalOutput").ap()

    with tile.TileContext(nc) as tc:
        with tc.tile_pool(name="big", bufs=3) as big, \
             tc.tile_pool(name="small", bufs=3) as small:
            off = 0
            mvoff = 0
            for D, m8 in sizes:
                _emit_bucket_ops(nc, (big, small), m_in, w_out, mv_out,
                                 D, m8, off, mvoff, d)
                off += m8 * d * D
                mvoff += m8
    nc.compile()
    return nc


_CACHE = {}


def _get_compiled(sizes, L, NV, d):
    key = (tuple(sizes), L, NV, d)
    if key not in _CACHE:
        _CACHE[key] = _build_bass(sizes, L, NV, d)
    return _CACHE[key]


def _unpack(results, sizes, meta, E, N, d, deg):
    M_v = np.full((N, d), -np.inf, dtype=np.float32)
    w = np.empty((E, d), dtype=np.float32)
    w_cores = [results[c]["w_out"] for c in range(N_CORES)]
    mv_cores = [results[c]["mv_out"] for c in range(N_CORES)]
    for (D, ids, eids, valid, off, mvoff) in meta:
        m = len(ids)
        m8 = m // N_CORES
        span = m8 * d * D
        wb = np.stack([w_cores[c][off:off + span] for c in range(N_CORES)])
        wb = wb.reshape(m, d, D).transpose(0, 2, 1)   # [m, D, d]
        w[eids[valid]] = wb[valid]
        mvb = np.stack(
            [mv_cores[c][mvoff * d:(mvoff + m8) * d] for c in range(N_CORES)]
        ).reshape(m, d)
        real = ids >= 0
        M_v[ids[real]] = mvb[real]
    return M_v, w.reshape(E, 1, d)


def kernel(M, dest, dim_size):
    M = np.ascontiguousarray(np.asarray(M, dtype=np.float32))
    dest = np.asarray(dest, dtype=np.int32)
    N = int(dim_size)
    E, d = M.shape

    flat, sizes, meta, L, NV, deg = _pack(M, dest, N)
    nc = _get_compiled(sizes, L, NV, d)
    in_maps = [{"m_in": flat[c]} for c in range(N_CORES)]
    res = bass_utils.run_bass_kernel_spmd(nc, in_maps,
                                          core_ids=list(range(N_CORES)))
    return _unpack(res.results, sizes, meta, E, N, d, deg)


# revision 13
# speedup vs baseline: 1.2393x; 1.2393x over previous
"""Trainium2 Bass kernel: fused segment_max + segment_softmax (GNN message passing).

Computes, for M [E, d] and dest [E] with N nodes:
  M_v = segment_max(M, dest, N)                         -> [N, d]
  w   = segment_softmax(M * BETA, dest)[:, None, :]     -> [E, 1, d]

Strategy: host groups edges by destination node (dest-based partitioning),
buckets nodes by degree (rounded up to a multiple of 4, padded with a large
negative value), and shards node groups evenly across the 8 NeuronCores so all
segment reductions are device-local. On device, nodes live one-per-partition
with their edges feature-major in the free dimension, so segment max/sum are
single Vector-engine reduces over the innermost axis and every DMA is a
contiguous 2D transfer.
"""

import numpy as np

import concourse.bacc as bacc
import concourse.tile as tile
from concourse import mybir
from concourse import bass_utils

BETA = 100.0
PAD_VAL = -1.0e30
N_CORES = 8
BUCKET_STEP = 1          # degree rounded up to multiple of this
TARGET_FREE = 4096       # target free-dim elements per super-tile (16 KiB/partition fp32)
ALIGN_NODES = N_CORES * 128  # bucket counts aligned to this so per-core tiles are full

# Engine balance: the 2 segment reduces must run on VectorE; the two
# broadcast tensor_tensor ops (subtract, multiply) can run on either VectorE
# or GpSimd. Values are the fraction of tiles whose op goes to GpSimd.
SUB_GP_FRAC = 0.0
MUL_GP_FRAC = 1.0
BIG_BUFS = 6             # slots in each big tile pool
SPLIT_BUF = False        # use separate input (mt) and z buffers per tile
RECIP_APPROX = True      # reciprocal_approx_accurate (~2 ULP) vs exact divide


def _build_plan(deg):
    """Group nodes into degree buckets.

    Returns list of (D, node_ids) where node_ids is an int64 array whose length
    is a multiple of N_CORES (padded with -1 = dummy node). All buckets except
    the last have length a multiple of ALIGN_NODES (via promotion of remainder
    nodes into the next bucket up), so per-core tiles use all 128 partitions.
    """
    N = deg.shape[0]
    nodes = np.flatnonzero(deg > 0)
    bucket_of = ((deg[nodes] + BUCKET_STEP - 1) // BUCKET_STEP) * BUCKET_STEP
    bucket_vals = np.unique(bucket_of)

    buckets = {}
    for D in bucket_vals:
        buckets[int(D)] = nodes[bucket_of == D]

    # Promote remainders upward so every bucket count is a multiple of
    # ALIGN_NODES; the last bucket keeps its remainder (padded to N_CORES).
    ds = sorted(buckets.keys())
    for i, D in enumerate(ds):
        if i == len(ds) - 1:
            break
        ids = buckets[D]
        rem = len(ids) % ALIGN_NODES
        if rem:
            nxt = ds[i + 1]
            buckets[nxt] = np.concatenate([ids[len(ids) - rem:], buckets[nxt]])
            buckets[D] = ids[: len(ids) - rem]

    plan = []
    for D in ds:
        ids = buckets[D]
        if len(ids) == 0:
            continue
        pad = (-len(ids)) % N_CORES
        if pad:
            ids = np.concatenate([ids, np.full(pad, -1, dtype=ids.dtype)])
        plan.append((int(D), ids))
    return plan


def _pack(M, dest, N):
    """Build per-core flat input arrays plus the metadata needed to unpack."""
    E, d = M.shape
    deg = np.bincount(dest, minlength=N)
    order = np.argsort(dest, kind="stable")
    starts = np.zeros(N + 1, dtype=np.int64)
    np.cumsum(deg, out=starts[1:])

    plan = _build_plan(deg)

    # Per-core flat layout: for each bucket (D, ids): per node [d, D]
    # feature-major. L = per-core element count.
    sizes = [(D, len(ids) // N_CORES) for D, ids in plan]
    L = sum(m8 * d * D for D, m8 in sizes)
    NV = sum(m8 for _, m8 in sizes)

    flat = np.full((N_CORES, L), PAD_VAL, dtype=np.float32)
    meta = []  # (D, ids [m], eids [m, D] int64, valid [m, D] bool, off, mvoff)
    off = 0
    mvoff = 0
    for D, ids in plan:
        m = len(ids)
        m8 = m // N_CORES
        safe_ids = np.where(ids >= 0, ids, 0)
        degs = np.where(ids >= 0, deg[safe_ids], 0)
        pos = starts[safe_ids][:, None] + np.arange(D)[None, :]
        valid = np.arange(D)[None, :] < degs[:, None]
        eids = order[np.minimum(pos, E - 1)]
        eids = np.where(valid, eids, 0)

        blk = M[eids]                      # [m, D, d]
        blk[~valid] = PAD_VAL
        blk = np.ascontiguousarray(blk.transpose(0, 2, 1))  # [m, d, D]
        span = m8 * d * D
        per_core = blk.reshape(N_CORES, span)
        flat[:, off:off + span] = per_core

        meta.append((D, ids, eids, valid, off, mvoff))
        off += span
        mvoff += m8
    assert off == L and mvoff == NV

    return flat, sizes, meta, L, NV, deg


def _emit_bucket_ops(nc, pools, m_in, w_out, mv_out, D, m8, off, mvoff, d,
                     counters):
    """Emit device ops for one degree bucket (per-core node count m8)."""
    big, small = pools
    A = max(1, TARGET_FREE // (d * D))
    f32 = mybir.dt.float32
    pos = 0
    while pos < m8:
        left = m8 - pos
        if left >= 128:
            a = min(A, left // 128)
            p = 128
            n_nodes = a * 128
        else:
            a = 1
            p = left
            n_nodes = left
        x = a * d          # node-feature columns in this tile
        fd = x * D         # free-dim elements in this tile

        base = off + pos * d * D
        mvbase = (mvoff + pos) * d

        mt = big.tile([128, fd], f32, tag="mt", name=f"mt_{D}_{pos}")
        nc.sync.dma_start(
            out=mt[:p, :],
            in_=m_in[base:base + p * fd].rearrange("(p x) -> p x", p=p),
        )
        mt3 = mt[:p, :].rearrange("p (x e) -> p x e", e=D)

        mx = small.tile([128, x], f32, tag="mx", name=f"mx_{D}_{pos}")
        nc.vector.tensor_reduce(
            out=mx[:p, :], in_=mt3, axis=mybir.AxisListType.X,
            op=mybir.AluOpType.max,
        )
        nc.sync.dma_start(
            out=mv_out[mvbase:mvbase + p * x].rearrange("(p x) -> p x", p=p),
            in_=mx[:p, :],
        )

        def tt(engine_frac, key, out, in0, in1, op):
            # Deterministically route this op to GpSimd for `engine_frac`
            # of tiles (fair interleave), VectorE otherwise.
            cnt, acc = counters[key]
            acc2 = acc + engine_frac
            use_gp = int(acc2) > int(acc + 1e-9)
            counters[key][0] = cnt + 1
            counters[key][1] = acc2
            eng = nc.gpsimd if use_gp else nc.vector
            eng.tensor_tensor(out=out, in0=in0, in1=in1, op=op)

        mx_b = mx[:p, :].unsqueeze(2).to_broadcast([p, x, D])
        if SPLIT_BUF:
            zt = big.tile([128, fd], f32, tag="zt", name=f"zt_{D}_{pos}")
            z3 = zt[:p, :].rearrange("p (x e) -> p x e", e=D)
            zf = zt[:p, :]
        else:
            z3 = mt3
            zf = mt[:p, :]
        tt(SUB_GP_FRAC, "sub", z3, mt3, mx_b, mybir.AluOpType.subtract)
        nc.scalar.activation(
            out=zf, in_=zf,
            func=mybir.ActivationFunctionType.Exp, scale=BETA,
        )
        sm = small.tile([128, x], f32, tag="sm", name=f"sm_{D}_{pos}")
        nc.vector.tensor_reduce(
            out=sm[:p, :], in_=z3, axis=mybir.AxisListType.X,
            op=mybir.AluOpType.add,
        )
        rc = small.tile([128, x], f32, tag="rc", name=f"rc_{D}_{pos}")
        if RECIP_APPROX:
            scr = small.tile([128, x], f32, tag="scr", name=f"scr_{D}_{pos}")
            nc.vector.reciprocal_approx_accurate(
                out=rc[:p, :], in_=sm[:p, :], scratch=scr[:p, :])
        else:
            nc.vector.reciprocal(out=rc[:p, :], in_=sm[:p, :])
        rc_b = rc[:p, :].unsqueeze(2).to_broadcast([p, x, D])
        tt(MUL_GP_FRAC, "mul", z3, z3, rc_b, mybir.AluOpType.mult)
        nc.sync.dma_start(
            out=w_out[base:base + p * fd].rearrange("(p x) -> p x", p=p),
            in_=zf,
        )
        pos += n_nodes


def _build_bass(sizes, L, NV, d):
    nc = bacc.Bacc("TRN2", target_bir_lowering=False, debug=False,
                   num_devices=N_CORES)
    f32 = mybir.dt.float32
    m_in = nc.dram_tensor("m_in", [L], f32, kind="ExternalInput").ap()
    w_out = nc.dram_tensor("w_out", [L], f32, kind="ExternalOutput").ap()
    mv_out = nc.dram_tensor("mv_out", [NV * d], f32, kind="ExternalOutput").ap()

    with tile.TileContext(nc) as tc:
        with tc.tile_pool(name="big", bufs=BIG_BUFS) as big, \
             tc.tile_pool(name="small", bufs=4) as small:
            off = 0
            mvoff = 0
            counters = {"sub": [0, 0.0], "mul": [0, 0.0]}
            for D, m8 in sizes:
                _emit_bucket_ops(nc, (big, small), m_in, w_out, mv_out,
                                 D, m8, off, mvoff, d, counters)
                off += m8 * d * D
                mvoff += m8
    nc.compile()
    return nc


_CACHE = {}


def _get_compiled(sizes, L, NV, d):
    key = (tuple(sizes), L, NV, d)
    if key not in _CACHE:
        _CACHE[key] = _build_bass(sizes, L, NV, d)
    return _CACHE[key]


def _unpack(results, sizes, meta, E, N, d, deg):
    M_v = np.full((N, d), -np.inf, dtype=np.float32)
    w = np.empty((E, d), dtype=np.float32)
    w_cores = [results[c]["w_out"] for c in range(N_CORES)]
    mv_cores = [results[c]["mv_out"] for c in range(N_CORES)]
    for (D, ids, eids, valid, off, mvoff) in meta:
        m = len(ids)
        m8 = m // N_CORES
        span = m8 * d * D
        wb = np.stack([w_cores[c][off:off + span] for c in range(N_CORES)])
        wb = wb.reshape(m, d, D).transpose(0, 2, 1)   # [m, D, d]
        w[eids[valid]] = wb[valid]
        mvb = np.stack(
            [mv_cores[c][mvoff * d:(mvoff + m8) * d] for c in range(N_CORES)]
        ).reshape(m, d)
        real = ids >= 0
        M_v[ids[real]] = mvb[real]
    return M_v, w.reshape(E, 1, d)


def kernel(M, dest, dim_size):
    M = np.ascontiguousarray(np.asarray(M, dtype=np.float32))
    dest = np.asarray(dest, dtype=np.int32)
    N = int(dim_size)
    E, d = M.shape

    flat, sizes, meta, L, NV, deg = _pack(M, dest, N)
    nc = _get_compiled(sizes, L, NV, d)
    in_maps = [{"m_in": flat[c]} for c in range(N_CORES)]
    res = bass_utils.run_bass_kernel_spmd(nc, in_maps,
                                          core_ids=list(range(N_CORES)))
    return _unpack(res.results, sizes, meta, E, N, d, deg)
